# revision 12
# baseline (speedup 1.0000x reference)
"""Trainium2 Bass kernel for nn_ODESurvMultiple (dense_mlp, 8-core data parallel).

reference math (per sample row x[256], scalar t):
  pi    = softmax(relu(x@W1p+b1p) @ W2p + b2p)                      [K=8]
  g     = x @ W1o[:-1] + b1o                                        [H=512]
  h_n   = relu(g + c_n * (t * w))     c_n=(1+u_n)/2, w=W1o[-1]      [15, 512]
  f_n   = softplus(h_n @ W2o + b2o)                                 [15, 8]
  pred  = (t/2) * sum_n W_n f_n                                     [8]
  preds = pi * (1 - exp(-pred))
returns (preds, pi)

Sharding: batch 16384 split 8 ways (2048/core), weights replicated.
On-chip layout is feature-major ("transposed"): activations [feature, batch]
so every matmul contracts along partitions without transposing h.
"""

import os
import sys

for _p in (
    "/root/.axon_site",
    "/root/.axon_site/_ro/trn_rl_repo",
    "/root/.axon_site/_ro/pypackages",
    "/opt/trn_rl_repo",
):
    if os.path.isdir(_p) and _p not in sys.path:
        sys.path.append(_p)

import numpy as np

import concourse.bass as bass
import concourse.mybir as mybir
import concourse.tile as tile
from concourse import bacc
from concourse.bass_utils import run_bass_kernel_spmd
from concourse.masks import make_identity

F32 = mybir.dt.float32
AX = mybir.AxisListType
OP = mybir.AluOpType
AF = mybir.ActivationFunctionType

N_CORES = 8
B_FULL, COV, H, K, NQ = 16384, 256, 512, 8, 15
B = B_FULL // N_CORES  # 2048 per core
TT, TS = 4, 512        # batch column tiles
C = H // 128           # 4 H-chunks
CIN = COV // 128       # 2 cov-chunks

_u64, _w64 = np.polynomial.legendre.leggauss(NQ)
_U32 = _u64.astype(np.float32)
_W32 = _w64.astype(np.float32)
CN = [float(np.float32(0.5) * (np.float32(1.0) + u)) for u in _U32]
WN = [float(w) for w in _W32]

# --- tuning knobs -----------------------------------------------------------
# Of the 60 (n, c) h-build units: how many go through the PE (identity-matmul
# add) route instead of the DVE scalar_tensor_tensor route.
X_PE = 20
# Of the stt-route units, how many do their relu on DVE (rest on ACT).
STT_RELU_DVE = 12
# Of the PE-route units, how many do their relu on DVE (rest on ACT).
PE_RELU_DVE = 0
# float32r (full-rate fp32 PE mode) per matmul group. fp32 plain runs at 1/4
# throughput on trn2; fp32r is full rate for moving dim >= 256.
R_MM1 = True
R_MM2O = True
R_MM2P = True
R_PRED = True
R_G2 = True
R_PEROUTE = True
# biases as k=1 PE matmuls (True) or fused into the DVE psum evacuation ops
BIAS_ON_PE = False
# ---------------------------------------------------------------------------
F32R = mybir.dt.float32r


def _routes():
    n_units = NQ * C
    pe = [
        ((i + 1) * X_PE) // n_units > (i * X_PE) // n_units for i in range(n_units)
    ]
    stt_idx = [i for i in range(n_units) if not pe[i]]
    pe_idx = [i for i in range(n_units) if pe[i]]
    relu_dve = set()
    for cnt, idx in ((STT_RELU_DVE, stt_idx), (PE_RELU_DVE, pe_idx)):
        m = max(1, len(idx))
        for j, i in enumerate(idx):
            if ((j + 1) * cnt) // m > (j * cnt) // m:
                relu_dve.add(i)
    return pe, relu_dve


def build_kernel():
    nc = bacc.Bacc("TRN2", target_bir_lowering=False, debug=False)

    x_d = nc.dram_tensor("x", [B, COV], F32, kind="ExternalInput").ap()
    t_d = nc.dram_tensor("t", [B], F32, kind="ExternalInput").ap()
    w1p_d = nc.dram_tensor("W1p", [COV, H], F32, kind="ExternalInput").ap()
    b1p_d = nc.dram_tensor("b1p", [H], F32, kind="ExternalInput").ap()
    w2p_d = nc.dram_tensor("W2p", [H, K], F32, kind="ExternalInput").ap()
    b2p_d = nc.dram_tensor("b2p", [K], F32, kind="ExternalInput").ap()
    w1o_d = nc.dram_tensor("W1o", [COV + 1, H], F32, kind="ExternalInput").ap()
    b1o_d = nc.dram_tensor("b1o", [H], F32, kind="ExternalInput").ap()
    w2o_d = nc.dram_tensor("W2o", [H, K], F32, kind="ExternalInput").ap()
    b2o_d = nc.dram_tensor("b2o", [K], F32, kind="ExternalInput").ap()
    preds_d = nc.dram_tensor("preds", [B, K], F32, kind="ExternalOutput").ap()
    pi_d = nc.dram_tensor("pi", [B, K], F32, kind="ExternalOutput").ap()

    pe_route, relu_dve = _routes()

    with tile.TileContext(nc) as tc:
        with (
            tc.tile_pool(name="pers", bufs=1) as pers,
            tc.tile_pool(name="ph", bufs=6) as ph,
            tc.tile_pool(name="pxin", bufs=3) as pxin,
            tc.tile_pool(name="pft", bufs=3) as pft,
            tc.tile_pool(name="psm", bufs=2) as psm,
            tc.tile_pool(name="pps", bufs=4, space="PSUM") as pps,
            tc.tile_pool(name="ppred", bufs=4, space="PSUM") as ppred,
        ):
            def pt(name, shape):
                return pers.tile(shape, F32, tag=name, name=name)

            # ---- persistent SBUF tiles ----
            ident128 = pt("ident128", [128, 128])
            ident8 = pt("ident8", [8, 8])
            wdiag = pt("wdiag", [128, NQ * 8])
            b2o_col4 = pt("b2o_col4", [128, 1])
            xT = [pt(f"xT{ci}", [128, B]) for ci in range(CIN)]
            g_sb = [pt(f"g{c}", [128, B]) for c in range(C)]
            g2_sb = [pt(f"G2{c}", [128, B]) for c in range(C)]
            w1p_sb = [pt(f"w1p{ci}", [128, H]) for ci in range(CIN)]
            w1o_sb = [pt(f"w1o{ci}", [128, H]) for ci in range(CIN)]
            w2p_sb = [pt(f"w2p{c}", [128, K]) for c in range(C)]
            w2o_pad = [pt(f"w2opad{c}", [128, 32]) for c in range(C)]
            t_row = pt("t_row", [1, B])
            w_row = pt("w_row", [1, H])
            ones_row = pt("ones_row", [1, TS])
            wsc_cur = pt("wsc_cur", [1, H])
            b1p_row = pt("b1p_row", [1, H])
            b1o_row = pt("b1o_row", [1, H])
            b2p_row = pt("b2p_row", [1, K])
            b1p_pc = pt("b1p_pc", [128, C])
            b1o_pc = pt("b1o_pc", [128, C])
            b2p_c8 = pt("b2p_c8", [8, 1])
            b2o_col = pt("b2o_col", [8, 1])
            t_bmaj = pt("t_bmaj", [128, B // 128])
            negthalf = pt("negthalf", [128, B // 128])
            logits_b = pt("logits_b", [128, B // 128 * K])
            e_b = pt("e_b", [128, B // 128 * K])
            sums = pt("sums", [128, B // 128])
            rec = pt("rec", [128, B // 128])
            pi_b = pt("pi_b", [128, B // 128 * K])
            pred_b = pt("pred_b", [128, B // 128 * K])
            eneg = pt("eneg", [128, B // 128 * K])
            cif_b = pt("cif_b", [128, B // 128 * K])
            preds_b = pt("preds_b", [128, B // 128 * K])

            # ---- constants ----
            make_identity(nc, ident128)
            make_identity(nc, ident8)
            for n in range(NQ):
                nc.vector.tensor_scalar_mul(
                    wdiag[0:8, n * 8 : (n + 1) * 8], ident8, WN[n]
                )
            for q in range(1, 4):
                # replicate the 8-row diag band to partition offsets 32/64/96
                nc.sync.dma_start(
                    out=wdiag[32 * q : 32 * q + 8, :], in_=wdiag[0:8, :]
                )
            nc.vector.memset(ones_row, 1.0)

            # ---- weight / small input DMAs ----
            for ci in range(CIN):
                nc.sync.dma_start(out=w1p_sb[ci], in_=w1p_d[ci * 128 : (ci + 1) * 128, :])
                nc.sync.dma_start(out=w1o_sb[ci], in_=w1o_d[ci * 128 : (ci + 1) * 128, :])
            for c in range(C):
                nc.sync.dma_start(out=w2p_sb[c], in_=w2p_d[c * 128 : (c + 1) * 128, :])
                nc.vector.memset(w2o_pad[c], 0.0)
                nc.sync.dma_start(
                    out=w2o_pad[c][:, 0:K], in_=w2o_d[c * 128 : (c + 1) * 128, :]
                )
            nc.sync.dma_start(out=t_row, in_=t_d.rearrange("(a b) -> a b", a=1))
            nc.sync.dma_start(out=w_row, in_=w1o_d[COV : COV + 1, :])
            nc.sync.dma_start(out=b1p_row, in_=b1p_d.rearrange("(a h) -> a h", a=1))
            nc.sync.dma_start(out=b1o_row, in_=b1o_d.rearrange("(a h) -> a h", a=1))
            nc.sync.dma_start(out=b2p_row, in_=b2p_d.rearrange("(a k) -> a k", a=1))
            nc.sync.dma_start(out=b2o_col, in_=b2o_d.rearrange("(k a) -> k a", a=1))
            nc.vector.memset(b2o_col4, 0.0)
            for q in range(4):
                nc.sync.dma_start(
                    out=b2o_col4[32 * q : 32 * q + 8, :],
                    in_=b2o_d.rearrange("(k a) -> k a", a=1),
                )
            nc.sync.dma_start(out=t_bmaj, in_=t_d.rearrange("(T p) -> p T", p=128))
            nc.vector.tensor_scalar_mul(negthalf, t_bmaj, -0.5)

            # ---- x load + transpose to xT ----
            for T in range(TT):
                for j in range(4):
                    r = (T * 4 + j) * 128
                    xin = pxin.tile([128, COV], F32, tag="xin", name=f"xin_{T}_{j}")
                    nc.sync.dma_start(out=xin, in_=x_d[r : r + 128, :])
                    for ci in range(CIN):
                        pst = pps.tile([128, 128], F32, tag="ps", name=f"pxt_{T}_{j}_{ci}")
                        nc.tensor.transpose(pst, xin[:, ci * 128 : (ci + 1) * 128], ident128)
                        nc.vector.tensor_copy(xT[ci][:, r : r + 128], pst)

            # ---- G2 = w (outer) t  (feature-major rank-1) ----
            for c in range(C):
                for s in range(TT):
                    g2p = pps.tile([128, TS], F32, tag="ps", name=f"g2p_{c}_{s}")
                    nc.tensor.matmul(
                        g2p,
                        w_row[:, c * 128 : (c + 1) * 128],
                        t_row[:, s * TS : (s + 1) * TS],
                        start=True,
                        stop=True,
                    )
                    nc.scalar.copy(g2_sb[c][:, s * TS : (s + 1) * TS], g2p)

            # ---- layer-1 matmuls (both nets), bias as k=1 matmul ----
            h1pT = [
                ph.tile([128, B], F32, tag="h", name=f"h1pT{c}") for c in range(C)
            ]
            for T in range(TT):
                for c in range(C):
                    cs = slice(c * 128, (c + 1) * 128)
                    bs = slice(T * TS, (T + 1) * TS)
                    pso = pps.tile([128, TS], F32, tag="ps", name=f"pso_{T}_{c}")
                    for ci in range(CIN):
                        nc.tensor.matmul(
                            pso, w1o_sb[ci][:, cs], xT[ci][:, bs], start=(ci == 0), stop=False
                        )
                    nc.tensor.matmul(pso, b1o_row[:, cs], ones_row, start=False, stop=True)
                    nc.vector.tensor_copy(g_sb[c][:, bs], pso)

                    psp = pps.tile([128, TS], F32, tag="ps", name=f"psp_{T}_{c}")
                    for ci in range(CIN):
                        nc.tensor.matmul(
                            psp, w1p_sb[ci][:, cs], xT[ci][:, bs], start=(ci == 0), stop=False
                        )
                    nc.tensor.matmul(psp, b1p_row[:, cs], ones_row, start=False, stop=True)
                    nc.scalar.activation(h1pT[c][:, bs], psp, AF.Relu)

            # ---- pi-head logits + transpose to batch-major ----
            for T in range(TT):
                bs = slice(T * TS, (T + 1) * TS)
                psl = pps.tile([8, TS], F32, tag="ps", name=f"psl_{T}")
                for c in range(C):
                    nc.tensor.matmul(
                        psl, w2p_sb[c], h1pT[c][:, bs], start=(c == 0), stop=False
                    )
                nc.tensor.matmul(psl, b2p_row, ones_row, start=False, stop=True)
                lgt = psm.tile([8, TS], F32, tag="lg", name=f"lgt_{T}")
                nc.vector.tensor_copy(lgt, psl)
                for j in range(4):
                    pst = pps.tile([128, 8], F32, tag="ps", name=f"plt_{T}_{j}")
                    nc.tensor.transpose(pst, lgt[:, j * 128 : (j + 1) * 128], ident8)
                    jj = T * 4 + j
                    nc.vector.tensor_copy(logits_b[:, jj * 8 : (jj + 1) * 8], pst)

            # ---- quadrature accumulators (persistent PSUM) ----
            pred_ps = [
                ppred.tile([8, TS], F32, tag="pred", name=f"pred_ps{T}")
                for T in range(TT)
            ]

            # ---- h build + layer-2 + softplus + quadrature sum ----
            for n in range(NQ):
                if any(pe_route[n * C + c] for c in range(C)):
                    nc.vector.tensor_scalar_mul(wsc_cur, w_row, CN[n])
                h_tiles = []
                for c in range(C):
                    i = n * C + c
                    ht = ph.tile([128, B], F32, tag="h", name=f"h_{n}_{c}")
                    if pe_route[i]:
                        for s in range(TT):
                            ss = slice(s * TS, (s + 1) * TS)
                            psh = pps.tile([128, TS], F32, tag="ps", name=f"psh_{n}_{c}_{s}")
                            nc.tensor.matmul(psh, ident128, g_sb[c][:, ss], start=True, stop=False)
                            nc.tensor.matmul(
                                psh,
                                wsc_cur[:, c * 128 : (c + 1) * 128],
                                t_row[:, ss],
                                start=False,
                                stop=True,
                            )
                            if i in relu_dve:
                                nc.vector.tensor_scalar_max(ht[:, ss], psh, 0.0)
                            else:
                                nc.scalar.activation(ht[:, ss], psh, AF.Relu)
                    else:
                        nc.vector.scalar_tensor_tensor(
                            out=ht,
                            in0=g2_sb[c],
                            scalar=CN[n],
                            in1=g_sb[c],
                            op0=OP.mult,
                            op1=OP.add,
                        )
                        if i in relu_dve:
                            nc.vector.tensor_scalar_max(ht, ht, 0.0)
                        else:
                            nc.scalar.activation(ht, ht, AF.Relu)
                    h_tiles.append(ht)
                # layer-2: pack the four T-blocks of f_n at partition offsets
                # 0/32/64/96 of one [128, 512] psum tile so the softplus
                # (exp+ln fallback) runs at full ACT width.
                psfp = pps.tile([128, TS], F32, tag="ps", name=f"psfp_{n}")
                for T in range(TT):
                    bs = slice(T * TS, (T + 1) * TS)
                    for c in range(C):
                        nc.tensor.matmul(
                            psfp[32 * T : 32 * T + 32, :],
                            w2o_pad[c],
                            h_tiles[c][:, bs],
                            start=(c == 0),
                            stop=(c == C - 1),
                            tile_position=(0, 32 * T),
                        )
                ep = pft.tile([128, TS], F32, tag="ep", name=f"ep_{n}")
                nc.scalar.activation(ep, psfp, AF.Exp, bias=b2o_col4)
                ftp = pft.tile([128, TS], F32, tag="ft", name=f"ft_{n}")
                nc.scalar.activation(ftp, ep, AF.Ln, bias=1.0)
                for T in range(TT):
                    nc.tensor.matmul(
                        pred_ps[T],
                        wdiag[32 * T : 32 * T + 8, n * 8 : (n + 1) * 8],
                        ftp[32 * T : 32 * T + 8, :],
                        start=(n == 0),
                        stop=(n == NQ - 1),
                        tile_position=(32 * T, 0),
                    )

            # ---- final: transpose pred, softmax, cif, outputs ----
            for T in range(TT):
                prT = psm.tile([8, TS], F32, tag="prT", name=f"prT_{T}")
                nc.vector.tensor_copy(prT, pred_ps[T])
                for j in range(4):
                    pst = pps.tile([128, 8], F32, tag="ps", name=f"ppt_{T}_{j}")
                    nc.tensor.transpose(pst, prT[:, j * 128 : (j + 1) * 128], ident8)
                    jj = T * 4 + j
                    nc.vector.tensor_copy(pred_b[:, jj * 8 : (jj + 1) * 8], pst)

            nc.scalar.activation(e_b, logits_b, AF.Exp)
            nc.vector.tensor_reduce(
                sums, e_b.rearrange("p (t k) -> p t k", k=8), axis=AX.X, op=OP.add
            )
            nc.vector.reciprocal(rec, sums)
            for jj in range(B // 128):
                nc.vector.tensor_scalar_mul(
                    pi_b[:, jj * 8 : (jj + 1) * 8],
                    e_b[:, jj * 8 : (jj + 1) * 8],
                    rec[:, jj : jj + 1],
                )
                nc.scalar.activation(
                    eneg[:, jj * 8 : (jj + 1) * 8],
                    pred_b[:, jj * 8 : (jj + 1) * 8],
                    AF.Exp,
                    scale=negthalf[:, jj : jj + 1],
                )
            nc.vector.tensor_scalar(cif_b, eneg, -1.0, 1.0, OP.mult, OP.add)
            nc.vector.tensor_tensor(out=preds_b, in0=cif_b, in1=pi_b, op=OP.mult)

            nc.sync.dma_start(
                out=preds_d.rearrange("(j p) k -> p j k", p=128),
                in_=preds_b.rearrange("p (j k) -> p j k", k=8),
            )
            nc.sync.dma_start(
                out=pi_d.rearrange("(j p) k -> p j k", p=128),
                in_=pi_b.rearrange("p (j k) -> p j k", k=8),
            )

    nc.compile()
    return nc


_NC = None


def _get_nc():
    global _NC
    if _NC is None:
        _NC = build_kernel()
    return _NC


def _shard_inputs(inputs):
    in_maps = []
    for i in range(N_CORES):
        sl = slice(i * B, (i + 1) * B)
        m = {
            "x": np.ascontiguousarray(np.asarray(inputs["x"], np.float32)[sl]),
            "t": np.ascontiguousarray(np.asarray(inputs["t"], np.float32)[sl]),
        }
        for k in ("W1p", "b1p", "W2p", "b2p", "W1o", "b1o", "W2o", "b2o"):
            m[k] = np.asarray(inputs[k], np.float32)
        in_maps.append(m)
    return in_maps


def kernel(**inputs):
    nc = _get_nc()
    in_maps = _shard_inputs(inputs)
    res = run_bass_kernel_spmd(nc, in_maps, core_ids=list(range(N_CORES)))
    preds = np.concatenate([res.results[i]["preds"] for i in range(N_CORES)], axis=0)
    pi = np.concatenate([res.results[i]["pi"] for i in range(N_CORES)], axis=0)
    return (preds, pi)


# revision 62
# speedup vs baseline: 92.4609x; 92.4609x over previous
"""Trainium2 Bass kernel for nn_ODESurvMultiple (dense_mlp, 8-core data parallel).

reference math (per sample row x[256], scalar t):
  pi    = softmax(relu(x@W1p+b1p) @ W2p + b2p)                      [K=8]
  g     = x @ W1o[:-1] + b1o                                        [H=512]
  h_n   = relu(g + c_n * (t * w))     c_n=(1+u_n)/2, w=W1o[-1]      [15, 512]
  f_n   = softplus(h_n @ W2o + b2o)                                 [15, 8]
  pred  = (t/2) * sum_n W_n f_n                                     [8]
  preds = pi * (1 - exp(-pred))
returns (preds, pi)

Sharding: batch 16384 split 8 ways (2048/core), weights replicated.
On-chip layout is feature-major ("transposed"): activations [feature, batch]
so every matmul contracts along partitions without transposing h.
"""

import os
import sys

for _p in (
    "/root/.axon_site",
    "/root/.axon_site/_ro/trn_rl_repo",
    "/root/.axon_site/_ro/pypackages",
    "/opt/trn_rl_repo",
):
    if os.path.isdir(_p) and _p not in sys.path:
        sys.path.append(_p)

import numpy as np

import concourse.bass as bass
import concourse.mybir as mybir
import concourse.tile as tile
from concourse import bacc
from concourse.bass_utils import run_bass_kernel_spmd
from concourse.masks import make_identity

# The act-table selector greedily takes the first set containing each
# function, ping-ponging exp_and_others <-> natural_log for our exp+ln
# softplus (30 reloads, ~40us). Prefer the combined set.
_orig_get_tables = bacc.get_activation_tables


def _tables_combined_first(arch):
    # IMPORTANT: set ORDER must stay identical to act_info.json (the emitted
    # act_func_set_id is a positional index). To steer the greedy selector to
    # the combined exp+ln set, hide exp/ln from the other sets instead.
    t = _orig_get_tables(arch)
    pref = "natural_log_exp_and_others"
    if pref not in t:
        return t
    exp_fn = mybir.ActivationFunctionType.Exp
    ln_fn = mybir.ActivationFunctionType.Ln
    out = {}
    for k, v in t.items():
        if k != pref and (exp_fn in v or ln_fn in v):
            v = {f for f in v if f not in (exp_fn, ln_fn)}
        out[k] = v
    return out


bacc.get_activation_tables = _tables_combined_first

F32 = mybir.dt.float32
AX = mybir.AxisListType
OP = mybir.AluOpType
AF = mybir.ActivationFunctionType

N_CORES = 8
B_FULL, COV, H, K, NQ = 16384, 256, 512, 8, 15
B = B_FULL // N_CORES  # 2048 per core
TT, TS = 4, 512        # batch column tiles
C = H // 128           # 4 H-chunks
CIN = COV // 128       # 2 cov-chunks

_u64, _w64 = np.polynomial.legendre.leggauss(NQ)
_U32 = _u64.astype(np.float32)
_W32 = _w64.astype(np.float32)
CN = [float(np.float32(0.5) * (np.float32(1.0) + u)) for u in _U32]
WN = [float(w) for w in _W32]

# --- tuning knobs -----------------------------------------------------------
# Of the 60 (n, c) h-build units: how many go through the PE (identity-matmul
# add) route instead of the DVE scalar_tensor_tensor route.
X_PE = 8
# Of the stt-route units, how many run ENTIRELY on GPSIMD (stt + relu).
STT_GPS = 0
# Of the remaining stt-route units, relu on DVE / GPSIMD (rest ACT).
STT_RELU_DVE = 10
STT_RELU_GPS = 0
# Of the PE-route units, how many do their relu on DVE (rest on ACT).
PE_RELU_DVE = 8
# float32r (full-rate fp32 PE mode) per matmul group. fp32 plain runs at 1/4
# throughput on trn2; fp32r is full rate for moving dim >= 256.
R_MM1 = False
R_MM2O = False
R_MM2P = False
R_PRED = False
R_G2 = False
R_PEROUTE = False
# biases as k=1 PE matmuls (True) or fused into the DVE psum evacuation ops
BIAS_ON_PE = False
# ---------------------------------------------------------------------------
F32R = mybir.dt.float32r


def _spread(idx, cnt, m=None):
    m = m or max(1, len(idx))
    return {i for j, i in enumerate(idx) if ((j + 1) * cnt) // m > (j * cnt) // m}


def _routes():
    n_units = NQ * C
    pe = [
        ((i + 1) * X_PE) // n_units > (i * X_PE) // n_units for i in range(n_units)
    ]
    stt_idx = [i for i in range(n_units) if not pe[i]]
    pe_idx = [i for i in range(n_units) if pe[i]]
    gps_full = _spread(stt_idx, STT_GPS)
    stt_idx = [i for i in stt_idx if i not in gps_full]
    relu_dve = _spread(stt_idx, STT_RELU_DVE) | _spread(pe_idx, PE_RELU_DVE)
    rest = [i for i in stt_idx if i not in relu_dve]
    relu_gps = _spread(rest, STT_RELU_GPS)
    return pe, relu_dve, relu_gps, gps_full


def build_kernel():
    nc = bacc.Bacc("TRN2", target_bir_lowering=False, debug=False)

    x_d = nc.dram_tensor("x", [B, COV], F32, kind="ExternalInput").ap()
    t_d = nc.dram_tensor("t", [B], F32, kind="ExternalInput").ap()
    w1p_d = nc.dram_tensor("W1p", [COV, H], F32, kind="ExternalInput").ap()
    b1p_d = nc.dram_tensor("b1p", [H], F32, kind="ExternalInput").ap()
    w2p_d = nc.dram_tensor("W2p", [H, K], F32, kind="ExternalInput").ap()
    b2p_d = nc.dram_tensor("b2p", [K], F32, kind="ExternalInput").ap()
    w1o_d = nc.dram_tensor("W1o", [COV + 1, H], F32, kind="ExternalInput").ap()
    b1o_d = nc.dram_tensor("b1o", [H], F32, kind="ExternalInput").ap()
    w2o_d = nc.dram_tensor("W2o", [H, K], F32, kind="ExternalInput").ap()
    b2o_d = nc.dram_tensor("b2o", [K], F32, kind="ExternalInput").ap()
    preds_d = nc.dram_tensor("preds", [B, K], F32, kind="ExternalOutput").ap()
    pi_d = nc.dram_tensor("pi", [B, K], F32, kind="ExternalOutput").ap()

    pe_route, relu_dve, relu_gps, gps_full = _routes()

    with tile.TileContext(nc) as tc:
        with (
            tc.tile_pool(name="pers", bufs=1) as pers,
            tc.tile_pool(name="ph", bufs=7) as ph,
            tc.tile_pool(name="pxin", bufs=3) as pxin,
            tc.tile_pool(name="pft", bufs=2) as pft,
            tc.tile_pool(name="psm", bufs=2) as psm,
            tc.tile_pool(name="pps", bufs=6, space="PSUM") as pps,
        ):
            def pt(name, shape, dt=F32):
                return pers.tile(shape, dt, tag=name, name=name)

            DT1 = F32R if R_MM1 else F32      # xT / W1 (layer-1 operands)
            DT2O = F32R if R_MM2O else F32    # h tiles / W2o
            DT2P = F32R if R_MM2P else F32    # h1pT / W2p
            DTPR = F32R if R_PRED else F32    # ftp / wdiag
            DTK1 = F32R if R_G2 else F32      # G2 rank-1 operands
            DTPE = F32R if R_PEROUTE else F32  # identity/g/wsc/t for PE h-route

            # ---- persistent SBUF tiles ----
            ident128 = pt("ident128", [128, 128])
            ident8 = pt("ident8", [8, 8])
            pred_sb = pt("pred_sb", [8, B])
            pred_sb2 = pt("pred_sb2", [8, B])
            b2o_col4 = pt("b2o_col4", [32, 1])
# xT lives in the h pool: consumed by mm1 early, slots then recycle for h

            g_sb = [pt(f"g{c}", [128, B], DTPE) for c in range(C)]
            g2_sb = [pt(f"G2{c}", [128, B]) for c in range(C)]
            w1p_sb = [pt(f"w1p{ci}", [128, H], DT1) for ci in range(CIN)]
            w1o_sb = [pt(f"w1o{ci}", [128, H], DT1) for ci in range(CIN)]
            w2p_sb = [pt(f"w2p{c}", [128, K], DT2P) for c in range(C)]
            w2o_pad = [pt(f"w2opad{c}", [128, 32], DT2O) for c in range(C)]
            identR = pt("identR", [128, 128], DTPE)
            t_row = pt("t_row", [1, B])
            t_row_r = pt("t_row_r", [1, B], DTK1)
            w_row = pt("w_row", [1, H])
            w_row_r = pt("w_row_r", [1, H], DTK1)
            ones_row = pt("ones_row", [1, TS])
            wsc_cur = pt("wsc_cur", [1, H], DTPE)
            b1p_row = pt("b1p_row", [1, H])
            b1o_row = pt("b1o_row", [1, H])
            b2p_row = pt("b2p_row", [1, K])
            b1p_pc = pt("b1p_pc", [128, C])
            b1o_pc = pt("b1o_pc", [128, C])
            b2p_c8 = pt("b2p_c8", [8, 1])
            b2o_col = pt("b2o_col", [8, 1])
            t_bmaj = pt("t_bmaj", [128, B // 128])
            negthalf = pt("negthalf", [128, B // 128])
            logits_b = pt("logits_b", [128, B // 128 * K])
            e_b = pt("e_b", [128, B // 128 * K])
            sums = pt("sums", [128, B // 128])
            rec = pt("rec", [128, B // 128])
            pi_b = pt("pi_b", [128, B // 128 * K])
            pred_b = pt("pred_b", [128, B // 128 * K])
            eneg = pt("eneg", [128, B // 128 * K])
            cif_b = pt("cif_b", [128, B // 128 * K])
            preds_b = pt("preds_b", [128, B // 128 * K])

            # ---- constants ----
            make_identity(nc, ident128)
            make_identity(nc, ident8)
            # fp32r matmul operands must come from a compute op that rounds
            # them, so every DMA-loaded matmul operand gets a DVE copy.
            nc.vector.tensor_copy(identR, ident128)
            nc.vector.memset(ones_row, 1.0)

            # ---- weight / small input DMAs (load fp32, round to fp32r) ----
            w1p_ld = [
                psm.tile([128, H], F32, tag="wld", name=f"w1pld{ci}")
                for ci in range(CIN)
            ]
            w1o_ld = [
                psm.tile([128, H], F32, tag="wld2", name=f"w1old{ci}")
                for ci in range(CIN)
            ]
            for ci in range(CIN):
                nc.sync.dma_start(out=w1p_ld[ci], in_=w1p_d[ci * 128 : (ci + 1) * 128, :])
                nc.sync.dma_start(out=w1o_ld[ci], in_=w1o_d[ci * 128 : (ci + 1) * 128, :])
                nc.vector.tensor_copy(w1p_sb[ci], w1p_ld[ci])
                nc.vector.tensor_copy(w1o_sb[ci], w1o_ld[ci])
            for c in range(C):
                w2p_ld = psm.tile([128, K], F32, tag="w2ld", name=f"w2pld{c}")
                nc.sync.dma_start(out=w2p_ld, in_=w2p_d[c * 128 : (c + 1) * 128, :])
                nc.vector.tensor_copy(w2p_sb[c], w2p_ld)
                w2o_ld = psm.tile([128, 32], F32, tag="w2ld2", name=f"w2old{c}")
                nc.vector.memset(w2o_ld, 0.0)
                nc.sync.dma_start(
                    out=w2o_ld[:, 0:K], in_=w2o_d[c * 128 : (c + 1) * 128, :]
                )
                nc.vector.tensor_copy(w2o_pad[c], w2o_ld)
            nc.sync.dma_start(out=t_row, in_=t_d.rearrange("(a b) -> a b", a=1))
            nc.sync.dma_start(out=w_row, in_=w1o_d[COV : COV + 1, :])
            nc.vector.tensor_copy(t_row_r, t_row)
            nc.vector.tensor_copy(w_row_r, w_row)
            nc.sync.dma_start(out=b1p_row, in_=b1p_d.rearrange("(a h) -> a h", a=1))
            nc.sync.dma_start(out=b1o_row, in_=b1o_d.rearrange("(a h) -> a h", a=1))
            nc.sync.dma_start(out=b2p_row, in_=b2p_d.rearrange("(a k) -> a k", a=1))
            nc.sync.dma_start(out=b1p_pc, in_=b1p_d.rearrange("(c p) -> p c", p=128))
            nc.sync.dma_start(out=b1o_pc, in_=b1o_d.rearrange("(c p) -> p c", p=128))
            nc.sync.dma_start(out=b2p_c8, in_=b2p_d.rearrange("(k a) -> k a", a=1))
            nc.sync.dma_start(out=b2o_col, in_=b2o_d.rearrange("(k a) -> k a", a=1))
            for q in range(4):
                nc.sync.dma_start(
                    out=b2o_col4[8 * q : 8 * q + 8, :],
                    in_=b2o_d.rearrange("(k a) -> k a", a=1),
                )
            nc.sync.dma_start(out=t_bmaj, in_=t_d.rearrange("(T p) -> p T", p=128))
            nc.vector.tensor_scalar_mul(negthalf, t_bmaj, -0.5)

            # ---- x load + transpose to xT ----
            xT = [
                ph.tile([128, B], DT1, tag="h", name=f"xT{ci}") for ci in range(CIN)
            ]
            for T in range(TT):
                for j in range(4):
                    r = (T * 4 + j) * 128
                    xin = pxin.tile([128, COV], F32, tag="xin", name=f"xin_{T}_{j}")
                    nc.sync.dma_start(out=xin, in_=x_d[r : r + 128, :])
                    for ci in range(CIN):
                        pst = pps.tile([128, 128], F32, tag="ps", name=f"pxt_{T}_{j}_{ci}")
                        nc.tensor.transpose(pst, xin[:, ci * 128 : (ci + 1) * 128], ident128)
                        nc.scalar.copy(xT[ci][:, r : r + 128], pst)

            # ---- G2 = w (outer) t  (feature-major rank-1) ----
            for c in range(C):
                for s in range(TT):
                    g2p = pps.tile([128, TS], F32, tag="ps", name=f"g2p_{c}_{s}")
                    nc.tensor.matmul(
                        g2p,
                        w_row_r[:, c * 128 : (c + 1) * 128],
                        t_row_r[:, s * TS : (s + 1) * TS],
                        start=True,
                        stop=True,
                    )
                    nc.scalar.copy(g2_sb[c][:, s * TS : (s + 1) * TS], g2p)

            # ---- layer-1 matmuls (both nets), bias as k=1 matmul ----
            h1pT = [
                ph.tile([128, B], DT2P, tag="h", name=f"h1pT{c}") for c in range(C)
            ]
            for T in range(TT):
                for c in range(C):
                    cs = slice(c * 128, (c + 1) * 128)
                    bs = slice(T * TS, (T + 1) * TS)
                    pso = pps.tile([128, TS], F32, tag="ps", name=f"pso_{T}_{c}")
                    for ci in range(CIN):
                        nc.tensor.matmul(
                            pso, w1o_sb[ci][:, cs], xT[ci][:, bs],
                            start=(ci == 0), stop=(ci == CIN - 1 and not BIAS_ON_PE)
                        )
                    if BIAS_ON_PE:
                        nc.tensor.matmul(pso, b1o_row[:, cs], ones_row, start=False, stop=True)
                        nc.vector.tensor_copy(g_sb[c][:, bs], pso)
                    else:
                        nc.vector.tensor_scalar_add(
                            g_sb[c][:, bs], pso, b1o_pc[:, c : c + 1]
                        )

                    psp = pps.tile([128, TS], F32, tag="ps", name=f"psp_{T}_{c}")
                    for ci in range(CIN):
                        nc.tensor.matmul(
                            psp, w1p_sb[ci][:, cs], xT[ci][:, bs],
                            start=(ci == 0), stop=(ci == CIN - 1 and not BIAS_ON_PE)
                        )
                    if BIAS_ON_PE:
                        nc.tensor.matmul(psp, b1p_row[:, cs], ones_row, start=False, stop=True)
                        nc.scalar.activation(h1pT[c][:, bs], psp, AF.Relu)
                    else:
                        # fused bias + relu in one DVE tensor_scalar
                        nc.vector.tensor_scalar(
                            h1pT[c][:, bs], psp, b1p_pc[:, c : c + 1], 0.0,
                            OP.add, OP.max,
                        )

            # ---- pi-head logits + transpose to batch-major ----
            for T in range(TT):
                bs = slice(T * TS, (T + 1) * TS)
                psl = pps.tile([8, TS], F32, tag="ps", name=f"psl_{T}")
                for c in range(C):
                    nc.tensor.matmul(
                        psl, w2p_sb[c], h1pT[c][:, bs],
                        start=(c == 0), stop=(c == C - 1 and not BIAS_ON_PE)
                    )
                lgt = psm.tile([8, TS], F32, tag="lg", name=f"lgt_{T}")
                if BIAS_ON_PE:
                    nc.tensor.matmul(psl, b2p_row, ones_row, start=False, stop=True)
                    nc.vector.tensor_copy(lgt, psl)
                else:
                    nc.vector.tensor_scalar_add(lgt, psl, b2p_c8)
                for j in range(4):
                    pst = pps.tile([128, 8], F32, tag="ps", name=f"plt_{T}_{j}")
                    nc.tensor.transpose(pst, lgt[:, j * 128 : (j + 1) * 128], ident8)
                    jj = T * 4 + j
                    nc.scalar.copy(logits_b[:, jj * 8 : (jj + 1) * 8], pst)



            # ---- h build + layer-2 + softplus + quadrature sum ----
            for n in range(NQ):
                if any(pe_route[n * C + c] for c in range(C)):
                    nc.vector.tensor_scalar_mul(wsc_cur, w_row, CN[n])
                h_tiles = []
                for c in range(C):
                    i = n * C + c
                    ht = ph.tile([128, B], DT2O, tag="h", name=f"h_{n}_{c}")
                    if pe_route[i]:
                        for s in range(TT):
                            ss = slice(s * TS, (s + 1) * TS)
                            psh = pps.tile([128, TS], F32, tag="ps", name=f"psh_{n}_{c}_{s}")
                            nc.tensor.matmul(psh, identR, g_sb[c][:, ss], start=True, stop=False)
                            nc.tensor.matmul(
                                psh,
                                wsc_cur[:, c * 128 : (c + 1) * 128],
                                t_row_r[:, ss],
                                start=False,
                                stop=True,
                            )
                            if i in relu_dve:
                                nc.vector.tensor_scalar_max(ht[:, ss], psh, 0.0)
                            else:
                                nc.scalar.activation(ht[:, ss], psh, AF.Relu)
                    else:
                        eng = nc.gpsimd if i in gps_full else nc.vector
                        eng.scalar_tensor_tensor(
                            out=ht,
                            in0=g2_sb[c],
                            scalar=CN[n],
                            in1=g_sb[c],
                            op0=OP.mult,
                            op1=OP.add,
                        )
                        if i in gps_full:
                            nc.gpsimd.tensor_scalar_max(ht, ht, 0.0)
                        elif i in relu_dve:
                            nc.vector.tensor_scalar_max(ht, ht, 0.0)
                        elif i in relu_gps:
                            nc.gpsimd.tensor_scalar_max(ht, ht, 0.0)
                        else:
                            nc.scalar.activation(ht, ht, AF.Relu)
                    h_tiles.append(ht)
                # layer-2 per T-block into [8,512] psum (fp32r forbids column
                # tile_position). The Exp of the softplus doubles as the psum
                # evacuation (bias=b2o fused); DMA packs the four exp'd bands
                # so the Ln runs once per n at 4x partition width.
                for T in range(TT):
                    bs = slice(T * TS, (T + 1) * TS)
                    psf = pps.tile([8, TS], F32, tag="ps", name=f"psf_{n}_{T}")
                    for c in range(C):
                        nc.tensor.matmul(
                            psf,
                            w2o_pad[c][:, 0:K],
                            h_tiles[c][:, bs],
                            start=(c == 0),
                            stop=(c == C - 1),
                        )
                    etmp = pft.tile([8, TS], F32, tag="et", name=f"et_{n}_{T}")
                    nc.scalar.activation(etmp, psf, AF.Exp, bias=b2o_col)
                    ftp = pft.tile([8, TS], DTPR, tag="ft", name=f"ft_{n}_{T}")
                    nc.scalar.activation(ftp, etmp, AF.Ln, bias=1.0)
                    # ping-pong accumulators: never read+write the same AP
                    src_t = pred_sb if n % 2 == 0 else pred_sb2
                    dst_t = pred_sb2 if n % 2 == 0 else pred_sb
                    if n == 0:
                        nc.vector.tensor_scalar_mul(dst_t[:, bs], ftp, WN[0])
                    else:
                        nc.vector.scalar_tensor_tensor(
                            out=dst_t[:, bs], in0=ftp, scalar=WN[n],
                            in1=src_t[:, bs], op0=OP.mult, op1=OP.add,
                        )

            # ---- final: transpose pred to batch-major, softmax ----
            for T in range(TT):
                for j in range(4):
                    pst = pps.tile([128, 8], F32, tag="ps", name=f"ppt_{T}_{j}")
                    nc.tensor.transpose(
                        pst,
                        pred_sb2[:, T * TS + j * 128 : T * TS + (j + 1) * 128],
                        ident8,
                    )
                    jj = T * 4 + j
                    nc.scalar.copy(pred_b[:, jj * 8 : (jj + 1) * 8], pst)

            nc.scalar.activation(e_b, logits_b, AF.Exp)
            nc.vector.tensor_reduce(
                sums, e_b.rearrange("p (t k) -> p t k", k=8), axis=AX.X, op=OP.add
            )
            nc.vector.reciprocal(rec, sums)
            for jj in range(B // 128):
                nc.vector.tensor_scalar_mul(
                    pi_b[:, jj * 8 : (jj + 1) * 8],
                    e_b[:, jj * 8 : (jj + 1) * 8],
                    rec[:, jj : jj + 1],
                )
                nc.scalar.activation(
                    eneg[:, jj * 8 : (jj + 1) * 8],
                    pred_b[:, jj * 8 : (jj + 1) * 8],
                    AF.Exp,
                    scale=negthalf[:, jj : jj + 1],
                )
            nc.vector.tensor_scalar(cif_b, eneg, -1.0, 1.0, OP.mult, OP.add)
            nc.vector.tensor_tensor(out=preds_b, in0=cif_b, in1=pi_b, op=OP.mult)

            nc.sync.dma_start(
                out=preds_d.rearrange("(j p) k -> p j k", p=128),
                in_=preds_b.rearrange("p (j k) -> p j k", k=8),
            )
            nc.sync.dma_start(
                out=pi_d.rearrange("(j p) k -> p j k", p=128),
                in_=pi_b.rearrange("p (j k) -> p j k", k=8),
            )

    nc.compile()
    return nc


_NC = None


def _get_nc():
    global _NC
    if _NC is None:
        _NC = build_kernel()
    return _NC


def _shard_inputs(inputs):
    in_maps = []
    for i in range(N_CORES):
        sl = slice(i * B, (i + 1) * B)
        m = {
            "x": np.ascontiguousarray(np.asarray(inputs["x"], np.float32)[sl]),
            "t": np.ascontiguousarray(np.asarray(inputs["t"], np.float32)[sl]),
        }
        for k in ("W1p", "b1p", "W2p", "b2p", "W1o", "b1o", "W2o", "b2o"):
            m[k] = np.asarray(inputs[k], np.float32)
        in_maps.append(m)
    return in_maps


def kernel(**inputs):
    nc = _get_nc()
    in_maps = _shard_inputs(inputs)
    res = run_bass_kernel_spmd(nc, in_maps, core_ids=list(range(N_CORES)))
    preds = np.concatenate([res.results[i]["preds"] for i in range(N_CORES)], axis=0)
    pi = np.concatenate([res.results[i]["pi"] for i in range(N_CORES)], axis=0)
    return (preds, pi)


# revision 63
# speedup vs baseline: 305.4898x; 3.3040x over previous
"""Trainium2 Bass kernel for nn_ODESurvMultiple (dense_mlp, 8-core data parallel).

reference math (per sample row x[256], scalar t):
  pi    = softmax(relu(x@W1p+b1p) @ W2p + b2p)                      [K=8]
  g     = x @ W1o[:-1] + b1o                                        [H=512]
  h_n   = relu(g + c_n * (t * w))     c_n=(1+u_n)/2, w=W1o[-1]      [15, 512]
  f_n   = softplus(h_n @ W2o + b2o)                                 [15, 8]
  pred  = (t/2) * sum_n W_n f_n                                     [8]
  preds = pi * (1 - exp(-pred))
returns (preds, pi)

Sharding: batch 16384 split 8 ways (2048/core), weights replicated.
On-chip layout is feature-major ("transposed"): activations [feature, batch]
so every matmul contracts along partitions without transposing h.
"""

import os
import sys

for _p in (
    "/root/.axon_site",
    "/root/.axon_site/_ro/trn_rl_repo",
    "/root/.axon_site/_ro/pypackages",
    "/opt/trn_rl_repo",
):
    if os.path.isdir(_p) and _p not in sys.path:
        sys.path.append(_p)

import numpy as np

import concourse.bass as bass
import concourse.mybir as mybir
import concourse.tile as tile
from concourse import bacc
from concourse.bass_utils import run_bass_kernel_spmd
from concourse.masks import make_identity

# The act-table selector greedily takes the first set containing each
# function, ping-ponging exp_and_others <-> natural_log for our exp+ln
# softplus (30 reloads, ~40us). Prefer the combined set.
_orig_get_tables = bacc.get_activation_tables


def _tables_combined_first(arch):
    # IMPORTANT: set ORDER must stay identical to act_info.json (the emitted
    # act_func_set_id is a positional index). To steer the greedy selector to
    # the combined exp+ln set, hide exp/ln from the other sets instead.
    t = _orig_get_tables(arch)
    pref = "natural_log_exp_and_others"
    if pref not in t:
        return t
    exp_fn = mybir.ActivationFunctionType.Exp
    ln_fn = mybir.ActivationFunctionType.Ln
    out = {}
    for k, v in t.items():
        if k != pref and (exp_fn in v or ln_fn in v):
            v = {f for f in v if f not in (exp_fn, ln_fn)}
        out[k] = v
    return out


bacc.get_activation_tables = _tables_combined_first

F32 = mybir.dt.float32
AX = mybir.AxisListType
OP = mybir.AluOpType
AF = mybir.ActivationFunctionType

N_CORES = 8
B_FULL, COV, H, K, NQ = 16384, 256, 512, 8, 15
B = B_FULL // N_CORES  # 2048 per core
TT, TS = 4, 512        # batch column tiles
C = H // 128           # 4 H-chunks
CIN = COV // 128       # 2 cov-chunks

_u64, _w64 = np.polynomial.legendre.leggauss(NQ)
_U32 = _u64.astype(np.float32)
_W32 = _w64.astype(np.float32)
CN = [float(np.float32(0.5) * (np.float32(1.0) + u)) for u in _U32]
WN = [float(w) for w in _W32]

# --- tuning knobs -----------------------------------------------------------
# Of the 60 (n, c) h-build units: how many go through the PE (identity-matmul
# add) route instead of the DVE scalar_tensor_tensor route.
X_PE = 30
# Of the stt-route units, how many run ENTIRELY on GPSIMD (stt + relu).
STT_GPS = 0
# Of the remaining stt-route units, relu on DVE / GPSIMD (rest ACT).
STT_RELU_DVE = 14
STT_RELU_GPS = 0
# Of the PE-route units, how many do their relu on DVE (rest on ACT).
PE_RELU_DVE = 16
# float32r (full-rate fp32 PE mode) per matmul group. fp32 plain runs at 1/4
# throughput on trn2; fp32r is full rate for moving dim >= 256.
R_MM1 = True
R_MM2O = True
R_MM2P = True
R_PRED = True
R_G2 = True
R_PEROUTE = True
# biases as k=1 PE matmuls (True) or fused into the DVE psum evacuation ops
BIAS_ON_PE = False
# ---------------------------------------------------------------------------
F32R = mybir.dt.float32r


def _spread(idx, cnt, m=None):
    m = m or max(1, len(idx))
    return {i for j, i in enumerate(idx) if ((j + 1) * cnt) // m > (j * cnt) // m}


def _routes():
    n_units = NQ * C
    pe = [
        ((i + 1) * X_PE) // n_units > (i * X_PE) // n_units for i in range(n_units)
    ]
    stt_idx = [i for i in range(n_units) if not pe[i]]
    pe_idx = [i for i in range(n_units) if pe[i]]
    gps_full = _spread(stt_idx, STT_GPS)
    stt_idx = [i for i in stt_idx if i not in gps_full]
    relu_dve = _spread(stt_idx, STT_RELU_DVE) | _spread(pe_idx, PE_RELU_DVE)
    rest = [i for i in stt_idx if i not in relu_dve]
    relu_gps = _spread(rest, STT_RELU_GPS)
    return pe, relu_dve, relu_gps, gps_full


def build_kernel():
    nc = bacc.Bacc("TRN2", target_bir_lowering=False, debug=False)

    x_d = nc.dram_tensor("x", [B, COV], F32, kind="ExternalInput").ap()
    t_d = nc.dram_tensor("t", [B], F32, kind="ExternalInput").ap()
    w1p_d = nc.dram_tensor("W1p", [COV, H], F32, kind="ExternalInput").ap()
    b1p_d = nc.dram_tensor("b1p", [H], F32, kind="ExternalInput").ap()
    w2p_d = nc.dram_tensor("W2p", [H, K], F32, kind="ExternalInput").ap()
    b2p_d = nc.dram_tensor("b2p", [K], F32, kind="ExternalInput").ap()
    w1o_d = nc.dram_tensor("W1o", [COV + 1, H], F32, kind="ExternalInput").ap()
    b1o_d = nc.dram_tensor("b1o", [H], F32, kind="ExternalInput").ap()
    w2o_d = nc.dram_tensor("W2o", [H, K], F32, kind="ExternalInput").ap()
    b2o_d = nc.dram_tensor("b2o", [K], F32, kind="ExternalInput").ap()
    preds_d = nc.dram_tensor("preds", [B, K], F32, kind="ExternalOutput").ap()
    pi_d = nc.dram_tensor("pi", [B, K], F32, kind="ExternalOutput").ap()

    pe_route, relu_dve, relu_gps, gps_full = _routes()

    with tile.TileContext(nc) as tc:
        with (
            tc.tile_pool(name="pers", bufs=1) as pers,
            tc.tile_pool(name="ph", bufs=7) as ph,
            tc.tile_pool(name="pxin", bufs=3) as pxin,
            tc.tile_pool(name="pft", bufs=2) as pft,
            tc.tile_pool(name="psm", bufs=2) as psm,
            tc.tile_pool(name="pps", bufs=6, space="PSUM") as pps,
        ):
            def pt(name, shape, dt=F32):
                return pers.tile(shape, dt, tag=name, name=name)

            DT1 = F32R if R_MM1 else F32      # xT / W1 (layer-1 operands)
            DT2O = F32R if R_MM2O else F32    # h tiles / W2o
            DT2P = F32R if R_MM2P else F32    # h1pT / W2p
            DTPR = F32R if R_PRED else F32    # ftp / wdiag
            DTK1 = F32R if R_G2 else F32      # G2 rank-1 operands
            DTPE = F32R if R_PEROUTE else F32  # identity/g/wsc/t for PE h-route

            # ---- persistent SBUF tiles ----
            ident128 = pt("ident128", [128, 128])
            ident8 = pt("ident8", [8, 8])
            pred_sb = pt("pred_sb", [8, B])
            pred_sb2 = pt("pred_sb2", [8, B])
            b2o_col4 = pt("b2o_col4", [32, 1])
# xT lives in the h pool: consumed by mm1 early, slots then recycle for h

            g_sb = [pt(f"g{c}", [128, B], DTPE) for c in range(C)]
            g2_sb = [pt(f"G2{c}", [128, B]) for c in range(C)]
            w1p_sb = [pt(f"w1p{ci}", [128, H], DT1) for ci in range(CIN)]
            w1o_sb = [pt(f"w1o{ci}", [128, H], DT1) for ci in range(CIN)]
            w2p_sb = [pt(f"w2p{c}", [128, K], DT2P) for c in range(C)]
            w2o_pad = [pt(f"w2opad{c}", [128, 32], DT2O) for c in range(C)]
            identR = pt("identR", [128, 128], DTPE)
            t_row = pt("t_row", [1, B])
            t_row_r = pt("t_row_r", [1, B], DTK1)
            w_row = pt("w_row", [1, H])
            w_row_r = pt("w_row_r", [1, H], DTK1)
            ones_row = pt("ones_row", [1, TS])
            wsc_cur = pt("wsc_cur", [1, H], DTPE)
            b1p_row = pt("b1p_row", [1, H])
            b1o_row = pt("b1o_row", [1, H])
            b2p_row = pt("b2p_row", [1, K])
            b1p_pc = pt("b1p_pc", [128, C])
            b1o_pc = pt("b1o_pc", [128, C])
            b2p_c8 = pt("b2p_c8", [8, 1])
            b2o_col = pt("b2o_col", [8, 1])
            t_bmaj = pt("t_bmaj", [128, B // 128])
            negthalf = pt("negthalf", [128, B // 128])
            logits_b = pt("logits_b", [128, B // 128 * K])
            e_b = pt("e_b", [128, B // 128 * K])
            sums = pt("sums", [128, B // 128])
            rec = pt("rec", [128, B // 128])
            pi_b = pt("pi_b", [128, B // 128 * K])
            pred_b = pt("pred_b", [128, B // 128 * K])
            eneg = pt("eneg", [128, B // 128 * K])
            cif_b = pt("cif_b", [128, B // 128 * K])
            preds_b = pt("preds_b", [128, B // 128 * K])

            # ---- constants ----
            make_identity(nc, ident128)
            make_identity(nc, ident8)
            # fp32r matmul operands must come from a compute op that rounds
            # them, so every DMA-loaded matmul operand gets a DVE copy.
            nc.vector.tensor_copy(identR, ident128)
            nc.vector.memset(ones_row, 1.0)

            # ---- weight / small input DMAs (load fp32, round to fp32r) ----
            w1p_ld = [
                psm.tile([128, H], F32, tag="wld", name=f"w1pld{ci}")
                for ci in range(CIN)
            ]
            w1o_ld = [
                psm.tile([128, H], F32, tag="wld2", name=f"w1old{ci}")
                for ci in range(CIN)
            ]
            for ci in range(CIN):
                nc.sync.dma_start(out=w1p_ld[ci], in_=w1p_d[ci * 128 : (ci + 1) * 128, :])
                nc.sync.dma_start(out=w1o_ld[ci], in_=w1o_d[ci * 128 : (ci + 1) * 128, :])
                nc.vector.tensor_copy(w1p_sb[ci], w1p_ld[ci])
                nc.vector.tensor_copy(w1o_sb[ci], w1o_ld[ci])
            for c in range(C):
                w2p_ld = psm.tile([128, K], F32, tag="w2ld", name=f"w2pld{c}")
                nc.sync.dma_start(out=w2p_ld, in_=w2p_d[c * 128 : (c + 1) * 128, :])
                nc.vector.tensor_copy(w2p_sb[c], w2p_ld)
                w2o_ld = psm.tile([128, 32], F32, tag="w2ld2", name=f"w2old{c}")
                nc.vector.memset(w2o_ld, 0.0)
                nc.sync.dma_start(
                    out=w2o_ld[:, 0:K], in_=w2o_d[c * 128 : (c + 1) * 128, :]
                )
                nc.vector.tensor_copy(w2o_pad[c], w2o_ld)
            nc.sync.dma_start(out=t_row, in_=t_d.rearrange("(a b) -> a b", a=1))
            nc.sync.dma_start(out=w_row, in_=w1o_d[COV : COV + 1, :])
            nc.vector.tensor_copy(t_row_r, t_row)
            nc.vector.tensor_copy(w_row_r, w_row)
            nc.sync.dma_start(out=b1p_row, in_=b1p_d.rearrange("(a h) -> a h", a=1))
            nc.sync.dma_start(out=b1o_row, in_=b1o_d.rearrange("(a h) -> a h", a=1))
            nc.sync.dma_start(out=b2p_row, in_=b2p_d.rearrange("(a k) -> a k", a=1))
            nc.sync.dma_start(out=b1p_pc, in_=b1p_d.rearrange("(c p) -> p c", p=128))
            nc.sync.dma_start(out=b1o_pc, in_=b1o_d.rearrange("(c p) -> p c", p=128))
            nc.sync.dma_start(out=b2p_c8, in_=b2p_d.rearrange("(k a) -> k a", a=1))
            nc.sync.dma_start(out=b2o_col, in_=b2o_d.rearrange("(k a) -> k a", a=1))
            for q in range(4):
                nc.sync.dma_start(
                    out=b2o_col4[8 * q : 8 * q + 8, :],
                    in_=b2o_d.rearrange("(k a) -> k a", a=1),
                )
            nc.sync.dma_start(out=t_bmaj, in_=t_d.rearrange("(T p) -> p T", p=128))
            nc.vector.tensor_scalar_mul(negthalf, t_bmaj, -0.5)

            # ---- x load + transpose to xT ----
            xT = [
                ph.tile([128, B], DT1, tag="h", name=f"xT{ci}") for ci in range(CIN)
            ]
            for T in range(TT):
                for j in range(4):
                    r = (T * 4 + j) * 128
                    xin = pxin.tile([128, COV], F32, tag="xin", name=f"xin_{T}_{j}")
                    nc.sync.dma_start(out=xin, in_=x_d[r : r + 128, :])
                    for ci in range(CIN):
                        pst = pps.tile([128, 128], F32, tag="ps", name=f"pxt_{T}_{j}_{ci}")
                        nc.tensor.transpose(pst, xin[:, ci * 128 : (ci + 1) * 128], ident128)
                        nc.scalar.copy(xT[ci][:, r : r + 128], pst)

            # ---- G2 = w (outer) t  (feature-major rank-1) ----
            for c in range(C):
                for s in range(TT):
                    g2p = pps.tile([128, TS], F32, tag="ps", name=f"g2p_{c}_{s}")
                    nc.tensor.matmul(
                        g2p,
                        w_row_r[:, c * 128 : (c + 1) * 128],
                        t_row_r[:, s * TS : (s + 1) * TS],
                        start=True,
                        stop=True,
                    )
                    nc.scalar.copy(g2_sb[c][:, s * TS : (s + 1) * TS], g2p)

            # ---- layer-1 matmuls (both nets), bias as k=1 matmul ----
            h1pT = [
                ph.tile([128, B], DT2P, tag="h", name=f"h1pT{c}") for c in range(C)
            ]
            for T in range(TT):
                for c in range(C):
                    cs = slice(c * 128, (c + 1) * 128)
                    bs = slice(T * TS, (T + 1) * TS)
                    pso = pps.tile([128, TS], F32, tag="ps", name=f"pso_{T}_{c}")
                    for ci in range(CIN):
                        nc.tensor.matmul(
                            pso, w1o_sb[ci][:, cs], xT[ci][:, bs],
                            start=(ci == 0), stop=(ci == CIN - 1 and not BIAS_ON_PE)
                        )
                    if BIAS_ON_PE:
                        nc.tensor.matmul(pso, b1o_row[:, cs], ones_row, start=False, stop=True)
                        nc.vector.tensor_copy(g_sb[c][:, bs], pso)
                    else:
                        nc.vector.tensor_scalar_add(
                            g_sb[c][:, bs], pso, b1o_pc[:, c : c + 1]
                        )

                    psp = pps.tile([128, TS], F32, tag="ps", name=f"psp_{T}_{c}")
                    for ci in range(CIN):
                        nc.tensor.matmul(
                            psp, w1p_sb[ci][:, cs], xT[ci][:, bs],
                            start=(ci == 0), stop=(ci == CIN - 1 and not BIAS_ON_PE)
                        )
                    if BIAS_ON_PE:
                        nc.tensor.matmul(psp, b1p_row[:, cs], ones_row, start=False, stop=True)
                        nc.scalar.activation(h1pT[c][:, bs], psp, AF.Relu)
                    else:
                        # fused bias + relu in one DVE tensor_scalar
                        nc.vector.tensor_scalar(
                            h1pT[c][:, bs], psp, b1p_pc[:, c : c + 1], 0.0,
                            OP.add, OP.max,
                        )

            # ---- pi-head logits + transpose to batch-major ----
            for T in range(TT):
                bs = slice(T * TS, (T + 1) * TS)
                psl = pps.tile([8, TS], F32, tag="ps", name=f"psl_{T}")
                for c in range(C):
                    nc.tensor.matmul(
                        psl, w2p_sb[c], h1pT[c][:, bs],
                        start=(c == 0), stop=(c == C - 1 and not BIAS_ON_PE)
                    )
                lgt = psm.tile([8, TS], F32, tag="lg", name=f"lgt_{T}")
                if BIAS_ON_PE:
                    nc.tensor.matmul(psl, b2p_row, ones_row, start=False, stop=True)
                    nc.vector.tensor_copy(lgt, psl)
                else:
                    nc.vector.tensor_scalar_add(lgt, psl, b2p_c8)
                for j in range(4):
                    pst = pps.tile([128, 8], F32, tag="ps", name=f"plt_{T}_{j}")
                    nc.tensor.transpose(pst, lgt[:, j * 128 : (j + 1) * 128], ident8)
                    jj = T * 4 + j
                    nc.scalar.copy(logits_b[:, jj * 8 : (jj + 1) * 8], pst)



            # ---- h build + layer-2 + softplus + quadrature sum ----
            for n in range(NQ):
                if any(pe_route[n * C + c] for c in range(C)):
                    nc.vector.tensor_scalar_mul(wsc_cur, w_row, CN[n])
                h_tiles = []
                for c in range(C):
                    i = n * C + c
                    ht = ph.tile([128, B], DT2O, tag="h", name=f"h_{n}_{c}")
                    if pe_route[i]:
                        for s in range(TT):
                            ss = slice(s * TS, (s + 1) * TS)
                            psh = pps.tile([128, TS], F32, tag="ps", name=f"psh_{n}_{c}_{s}")
                            nc.tensor.matmul(psh, identR, g_sb[c][:, ss], start=True, stop=False)
                            nc.tensor.matmul(
                                psh,
                                wsc_cur[:, c * 128 : (c + 1) * 128],
                                t_row_r[:, ss],
                                start=False,
                                stop=True,
                            )
                            if i in relu_dve:
                                nc.vector.tensor_scalar_max(ht[:, ss], psh, 0.0)
                            else:
                                nc.scalar.activation(ht[:, ss], psh, AF.Relu)
                    else:
                        eng = nc.gpsimd if i in gps_full else nc.vector
                        eng.scalar_tensor_tensor(
                            out=ht,
                            in0=g2_sb[c],
                            scalar=CN[n],
                            in1=g_sb[c],
                            op0=OP.mult,
                            op1=OP.add,
                        )
                        if i in gps_full:
                            nc.gpsimd.tensor_scalar_max(ht, ht, 0.0)
                        elif i in relu_dve:
                            nc.vector.tensor_scalar_max(ht, ht, 0.0)
                        elif i in relu_gps:
                            nc.gpsimd.tensor_scalar_max(ht, ht, 0.0)
                        else:
                            nc.scalar.activation(ht, ht, AF.Relu)
                    h_tiles.append(ht)
                # layer-2 per T-block into [8,512] psum (fp32r forbids column
                # tile_position). The Exp of the softplus doubles as the psum
                # evacuation (bias=b2o fused); DMA packs the four exp'd bands
                # so the Ln runs once per n at 4x partition width.
                for T in range(TT):
                    bs = slice(T * TS, (T + 1) * TS)
                    psf = pps.tile([8, TS], F32, tag="ps", name=f"psf_{n}_{T}")
                    for c in range(C):
                        nc.tensor.matmul(
                            psf,
                            w2o_pad[c][:, 0:K],
                            h_tiles[c][:, bs],
                            start=(c == 0),
                            stop=(c == C - 1),
                        )
                    etmp = pft.tile([8, TS], F32, tag="et", name=f"et_{n}_{T}")
                    nc.scalar.activation(etmp, psf, AF.Exp, bias=b2o_col)
                    ftp = pft.tile([8, TS], DTPR, tag="ft", name=f"ft_{n}_{T}")
                    nc.scalar.activation(ftp, etmp, AF.Ln, bias=1.0)
                    # ping-pong accumulators: never read+write the same AP
                    src_t = pred_sb if n % 2 == 0 else pred_sb2
                    dst_t = pred_sb2 if n % 2 == 0 else pred_sb
                    if n == 0:
                        nc.vector.tensor_scalar_mul(dst_t[:, bs], ftp, WN[0])
                    else:
                        nc.vector.scalar_tensor_tensor(
                            out=dst_t[:, bs], in0=ftp, scalar=WN[n],
                            in1=src_t[:, bs], op0=OP.mult, op1=OP.add,
                        )

            # ---- final: transpose pred to batch-major, softmax ----
            for T in range(TT):
                for j in range(4):
                    pst = pps.tile([128, 8], F32, tag="ps", name=f"ppt_{T}_{j}")
                    nc.tensor.transpose(
                        pst,
                        pred_sb2[:, T * TS + j * 128 : T * TS + (j + 1) * 128],
                        ident8,
                    )
                    jj = T * 4 + j
                    nc.scalar.copy(pred_b[:, jj * 8 : (jj + 1) * 8], pst)

            nc.scalar.activation(e_b, logits_b, AF.Exp)
            nc.vector.tensor_reduce(
                sums, e_b.rearrange("p (t k) -> p t k", k=8), axis=AX.X, op=OP.add
            )
            nc.vector.reciprocal(rec, sums)
            for jj in range(B // 128):
                nc.vector.tensor_scalar_mul(
                    pi_b[:, jj * 8 : (jj + 1) * 8],
                    e_b[:, jj * 8 : (jj + 1) * 8],
                    rec[:, jj : jj + 1],
                )
                nc.scalar.activation(
                    eneg[:, jj * 8 : (jj + 1) * 8],
                    pred_b[:, jj * 8 : (jj + 1) * 8],
                    AF.Exp,
                    scale=negthalf[:, jj : jj + 1],
                )
            nc.vector.tensor_scalar(cif_b, eneg, -1.0, 1.0, OP.mult, OP.add)
            nc.vector.tensor_tensor(out=preds_b, in0=cif_b, in1=pi_b, op=OP.mult)

            nc.sync.dma_start(
                out=preds_d.rearrange("(j p) k -> p j k", p=128),
                in_=preds_b.rearrange("p (j k) -> p j k", k=8),
            )
            nc.sync.dma_start(
                out=pi_d.rearrange("(j p) k -> p j k", p=128),
                in_=pi_b.rearrange("p (j k) -> p j k", k=8),
            )

    nc.compile()
    return nc


_NC = None


def _get_nc():
    global _NC
    if _NC is None:
        _NC = build_kernel()
    return _NC


def _shard_inputs(inputs):
    in_maps = []
    for i in range(N_CORES):
        sl = slice(i * B, (i + 1) * B)
        m = {
            "x": np.ascontiguousarray(np.asarray(inputs["x"], np.float32)[sl]),
            "t": np.ascontiguousarray(np.asarray(inputs["t"], np.float32)[sl]),
        }
        for k in ("W1p", "b1p", "W2p", "b2p", "W1o", "b1o", "W2o", "b2o"):
            m[k] = np.asarray(inputs[k], np.float32)
        in_maps.append(m)
    return in_maps


def kernel(**inputs):
    nc = _get_nc()
    in_maps = _shard_inputs(inputs)
    res = run_bass_kernel_spmd(nc, in_maps, core_ids=list(range(N_CORES)))
    preds = np.concatenate([res.results[i]["preds"] for i in range(N_CORES)], axis=0)
    pi = np.concatenate([res.results[i]["pi"] for i in range(N_CORES)], axis=0)
    return (preds, pi)
